# revision 18
# baseline (speedup 1.0000x reference)
"""Trainium2 Bass kernel for nn_GemNetOutput (segment_reduce + FiLM + MLP head).

Reference computation (all fp32):
    g     = segment_sum(x, batch, num_segments=B)        # [B, H]
    gamma = domain_emb @ gamma_w.T + gamma_b             # [B, H]
    beta  = domain_emb @ beta_w.T  + beta_b              # [B, H]
    g     = gamma * g + beta
    h     = silu(g @ w1.T + b1)                          # [B, H]
    h     = silu(h @ w2.T + b2)                          # [B, H/2]
    out   = (h @ w3.T + b3).squeeze(-1)                  # [B]

Shapes: N=1e6 nodes, B=16384 graphs, H=512, FD=16.  `batch` is SORTED.

Strategy (8 NeuronCores, no collectives needed):
  - The 16384 segments are BIN-PACKED into 128 windows of exactly 128
    segments each, equalizing the node count per window (greedy LPT on the
    host).  Core c owns windows [c*16, (c+1)*16).  All cores run one
    identical static program; every window is padded to the same t_tiles
    node tiles (sentinel one-hot ids mask the padding) -- balancing makes
    that padding ~1.5% instead of ~5%.
  - x is converted to fp8-e4m3 on the host with SIGMA-DELTA (error-feedback)
    rounding along each segment: quantization errors telescope within a
    segment, so the device's segment sums match the fp32 sums to ~1 quantum
    instead of sqrt(n) quanta.  Halves HBM traffic vs bf16.
  - x is packed on the host into [window, 128, t_tiles*H] so each DMA block
    is a fully contiguous ~1 MB transfer (8 KB per partition line).
  - segment_sum on the PE with fp8 DoubleRow matmuls: each matmul consumes
    TWO 128-node tiles (contraction 256) against a [128, 2, 128] one-hot.
  - One-hot built on DVE with one batched tensor_tensor per DMA block
    (stride-0 broadcast APs) instead of one tensor_scalar per tile.
  - beta (incl. beta_b) is folded into the MLP-1 accumulation on the host:
    ph1 += (W1 @ beta_w_ext.T) @ dom_ext, one small K=17 matmul per j-chunk.
  - FiLM multiply + MLP run per GROUP of 4 windows in transposed
    [feature, seg] layout so MLP matmuls have N=512 moving operands.
"""

import sys
from contextlib import ExitStack

for _p in ("/opt/trn_rl_repo", "/opt/pypackages"):
    if _p not in sys.path:
        sys.path.append(_p)

import ml_dtypes
import numpy as np

import concourse.bass as bass
import concourse.tile as tile
from concourse import bacc, mybir
from concourse import bass_utils

dt = mybir.dt

# Problem constants (hardcoded per the contract).
N_NODES = 1_000_000
B_SEGS = 16_384
H = 512
H2 = 256
FD = 16
N_CORES = 8
SEG_W = 128          # segments per window (PSUM partition dim)
GRP = 4              # windows per MLP group (moving N = GRP*SEG_W = 512)
XT = 32              # max node subtiles (of 128 rows) per x DMA block

BF16 = ml_dtypes.bfloat16
F8 = ml_dtypes.float8_e4m3fn

# CoreSim has no Silu LUT; compose silu = z * sigmoid(z) when True (sim tests).
SILU_COMPOSE = False


def _f32_to_bf16(a: np.ndarray) -> np.ndarray:
    return np.ascontiguousarray(a, dtype=np.float32).astype(BF16)


def _blocks_of(t_tiles: int) -> tuple:
    """Split t_tiles into even-sized DMA blocks of at most XT subtiles."""
    blocks = [XT] * (t_tiles // XT)
    rem = t_tiles % XT
    if rem:
        blocks.append(rem)
    assert all(b % 2 == 0 for b in blocks)
    return tuple(blocks)


def build_program(spc: int, t_tiles: int, n_cores: int):
    """Build the per-core Bass/Tile program.

    spc: segments per core (multiple of 128)
    t_tiles: node tiles (of 128) per 128-segment window; even
    """
    assert t_tiles % 2 == 0
    windows = spc // SEG_W
    blocks = _blocks_of(t_tiles)
    x_dt = dt.float8e4
    m_dt = dt.bfloat16             # MLP matmul dtype

    nc = bacc.Bacc(
        "TRN2",
        target_bir_lowering=False,
        debug=False,
        enable_asserts=False,
        num_devices=n_cores,
    )

    xp = nc.dram_tensor("xp", [windows, 128, t_tiles * H], x_dt,
                        kind="ExternalInput").ap()
    brt = nc.dram_tensor("brt", [128, windows * t_tiles], dt.bfloat16,
                         kind="ExternalInput").ap()
    domE = nc.dram_tensor("domE", [FD + 1, spc], m_dt, kind="ExternalInput").ap()
    gw = nc.dram_tensor("gw", [FD + 1, H], m_dt, kind="ExternalInput").ap()
    w1bw = nc.dram_tensor("w1bw", [FD + 1, H], m_dt, kind="ExternalInput").ap()
    w1t = nc.dram_tensor("w1t", [H, H], m_dt, kind="ExternalInput").ap()
    w2t = nc.dram_tensor("w2t", [H, H2], m_dt, kind="ExternalInput").ap()
    w3c = nc.dram_tensor("w3c", [128, H2 // 128], m_dt, kind="ExternalInput").ap()
    b1c = nc.dram_tensor("b1c", [128, H // 128], dt.float32, kind="ExternalInput").ap()
    b2c = nc.dram_tensor("b2c", [128, H2 // 128], dt.float32, kind="ExternalInput").ap()
    b3c = nc.dram_tensor("b3c", [1, 1], dt.float32, kind="ExternalInput").ap()
    iden = nc.dram_tensor("iden", [128, 128], dt.float32, kind="ExternalInput").ap()
    iotr = nc.dram_tensor("iotr", [128, 128], dt.bfloat16, kind="ExternalInput").ap()
    out = nc.dram_tensor("out", [1, spc], dt.float32, kind="ExternalOutput").ap()

    HC = H // 128       # 4 h-chunks
    JC = H // 128       # 4 layer-1 output chunks
    KC = H2 // 128      # 2 layer-2 output chunks
    NG = GRP * SEG_W    # max moving width of group-level MLP matmuls

    # group sizes: 4-window groups, but finish with 2/1/1 so the final MLP
    # chains are short and overlap the x DMA stream instead of trailing it
    gplan = []
    rem = windows
    while rem > 4:
        gplan.append(4)
        rem -= 4
    gplan.extend([2, 1, 1] if rem == 4 else [1] * rem)
    gstarts = [sum(gplan[:i]) for i in range(len(gplan))]
    group_of = {}
    for gi, (wg, gsz) in enumerate(zip(gstarts, gplan)):
        for w in range(wg, wg + gsz):
            group_of[w] = gi

    is_eq = mybir.AluOpType.is_equal
    DR = mybir.MatmulPerfMode.DoubleRow

    with tile.TileContext(nc) as tc, ExitStack() as ctx:
        cpool = ctx.enter_context(tc.tile_pool(name="consts", bufs=1))
        xpool = ctx.enter_context(tc.tile_pool(name="x", bufs=6))
        ohpool = ctx.enter_context(tc.tile_pool(name="oh", bufs=4))
        spool = ctx.enter_context(tc.tile_pool(name="work", bufs=2))
        pg = ctx.enter_context(tc.tile_pool(name="pg", bufs=2, space=bass.MemorySpace.PSUM))
        pt = ctx.enter_context(tc.tile_pool(name="pt", bufs=2, space=bass.MemorySpace.PSUM))
        pm = ctx.enter_context(tc.tile_pool(name="pm", bufs=3, space=bass.MemorySpace.PSUM))

        # ---- constants / weights into SBUF ----
        warm_sb = cpool.tile([128, 128], dt.bfloat16)
        nc.vector.memset(warm_sb[:], 1.0)
        iotr_sb = cpool.tile([128, 128], dt.bfloat16)
        nc.sync.dma_start(iotr_sb[:], iotr)
        iden_sb = cpool.tile([128, 128], dt.float32)
        w1_sb = cpool.tile([128, HC, H], m_dt)
        w2_sb = cpool.tile([128, HC, H2], m_dt)
        w3_sb = cpool.tile([128, KC], m_dt)
        b1_sb = cpool.tile([128, JC], dt.float32)
        b2_sb = cpool.tile([128, KC], dt.float32)
        b3_sb = cpool.tile([1, 1], dt.float32)

        def emit_weight_dmas():
            nc.sync.dma_start(iden_sb[:], iden)
            nc.sync.dma_start(w1_sb[:], w1t.rearrange("(c p) j -> p c j", p=128))
            nc.sync.dma_start(w2_sb[:], w2t.rearrange("(c p) j -> p c j", p=128))
            nc.sync.dma_start(w3_sb[:], w3c)
            nc.sync.dma_start(b1_sb[:], b1c)
            nc.sync.dma_start(b2_sb[:], b2c)
            nc.sync.dma_start(b3_sb[:], b3c)
        brt_sb = cpool.tile([128, windows * t_tiles], dt.bfloat16)
        nc.sync.dma_start(brt_sb[:], brt)
        gw_sb = cpool.tile([FD + 1, H], m_dt)
        nc.sync.dma_start(gw_sb[:], gw)
        w1bw_sb = cpool.tile([FD + 1, H], m_dt)
        nc.sync.dma_start(w1bw_sb[:], w1bw)
        domE_sb = cpool.tile([FD + 1, spc], m_dt)
        nc.sync.dma_start(domE_sb[:], domE)
        out_sb = cpool.tile([1, spc], dt.float32)

        # ---- PE warm-up: ~4.5us of dummy matmuls on a memset tile (no DMA
        # dependency, starts immediately) so HAM flips to K=8/8 before the
        # real stream starts.
        warm_t = pm.tile([128, H], dt.float32, tag="pmlp")
        for i in range(40):
            nc.tensor.matmul(
                warm_t[:, 0:128], warm_sb[:], warm_sb[:],
                start=(i == 0), stop=(i == 39))

        gstate = {}

        def emit_gamma(gi):
            """gammaT for group gi: [128 h, HC, span] bf16."""
            wg, span = gstarts[gi], gplan[gi] * SEG_W
            gam = spool.tile([128, HC, NG], m_dt, tag="gam")
            dom_s = domE_sb[:, wg * SEG_W: wg * SEG_W + span]
            for hc in range(HC):
                pgb = pm.tile([128, H], dt.float32, tag="pmlp")
                nc.tensor.matmul(
                    pgb[:, 0:span],
                    gw_sb[:, hc * 128:(hc + 1) * 128], dom_s,
                    start=True, stop=True)
                nc.scalar.copy(gam[:, hc, 0:span], pgb[:, 0:span])
            gstate[gi] = (gam, spool.tile(
                [128, HC, NG], m_dt, tag="gmodT", name="gmodT"))

        mlp_pieces = []   # queued thunks, drained a few per window

        def queue_mlp(wg, span, gi):
            """Queue the MLP for group gi as independently-emittable pieces."""
            gmodT = gstate[gi][1]
            dom_s = domE_sb[:, wg * SEG_W: wg * SEG_W + span]
            st = {}

            def mk_l1(jc):
                def piece():
                    if "h1" not in st:
                        st["h1"] = spool.tile(
                            [128, HC, NG], m_dt, tag="h1", name="h1")
                    h1 = st["h1"]
                    ph1 = pm.tile([128, NG], dt.float32, tag="pmlp")
                    nc.tensor.matmul(
                        ph1[:, 0:span],
                        w1bw_sb[:, jc * 128:(jc + 1) * 128], dom_s,
                        start=True, stop=False, skip_group_check=True)
                    for hc in range(HC):
                        nc.tensor.matmul(
                            ph1[:, 0:span],
                            w1_sb[:, hc, jc * 128:(jc + 1) * 128],
                            gmodT[:, hc, 0:span],
                            start=False, stop=(hc == HC - 1),
                            skip_group_check=True)
                    if SILU_COMPOSE:
                        z1 = spool.tile([128, NG], dt.float32, tag="z1")
                        nc.scalar.activation(
                            z1[:, 0:span], ph1[:, 0:span],
                            mybir.ActivationFunctionType.Identity,
                            bias=b1_sb[:, jc:jc + 1])
                        nc.scalar.activation(
                            h1[:, jc, 0:span], z1[:, 0:span],
                            mybir.ActivationFunctionType.Sigmoid)
                        nc.vector.tensor_mul(
                            h1[:, jc, 0:span], h1[:, jc, 0:span], z1[:, 0:span])
                    else:
                        nc.scalar.activation(
                            h1[:, jc, 0:span], ph1[:, 0:span],
                            mybir.ActivationFunctionType.Silu,
                            bias=b1_sb[:, jc:jc + 1])
                return piece

            def mk_l2(kc):
                def piece():
                    if "h2" not in st:
                        st["h2"] = spool.tile(
                            [128, KC, NG], m_dt, tag="h2", name="h2")
                    h1, h2 = st["h1"], st["h2"]
                    ph2 = pm.tile([128, NG], dt.float32, tag="pmlp")
                    for hc in range(HC):
                        nc.tensor.matmul(
                            ph2[:, 0:span],
                            w2_sb[:, hc, kc * 128:(kc + 1) * 128],
                            h1[:, hc, 0:span],
                            start=(hc == 0), stop=(hc == HC - 1))
                    if SILU_COMPOSE:
                        z2 = spool.tile([128, NG], dt.float32, tag="z2")
                        nc.scalar.activation(
                            z2[:, 0:span], ph2[:, 0:span],
                            mybir.ActivationFunctionType.Identity,
                            bias=b2_sb[:, kc:kc + 1])
                        nc.scalar.activation(
                            h2[:, kc, 0:span], z2[:, 0:span],
                            mybir.ActivationFunctionType.Sigmoid)
                        nc.vector.tensor_mul(
                            h2[:, kc, 0:span], h2[:, kc, 0:span], z2[:, 0:span])
                    else:
                        nc.scalar.activation(
                            h2[:, kc, 0:span], ph2[:, 0:span],
                            mybir.ActivationFunctionType.Silu,
                            bias=b2_sb[:, kc:kc + 1])
                return piece

            def head():
                h2 = st["h2"]
                po = pm.tile([1, NG], dt.float32, tag="pmlp")
                for kc in range(KC):
                    nc.tensor.matmul(
                        po[:, 0:span], w3_sb[:, kc:kc + 1], h2[:, kc, 0:span],
                        start=(kc == 0), stop=(kc == KC - 1))
                nc.scalar.activation(
                    out_sb[0:1, wg * SEG_W: wg * SEG_W + span], po[:, 0:span],
                    mybir.ActivationFunctionType.Identity,
                    bias=b3_sb[0:1, 0:1])
                nc.sync.dma_start(
                    out[0:1, wg * SEG_W: wg * SEG_W + span],
                    out_sb[0:1, wg * SEG_W: wg * SEG_W + span])

            for jc in range(JC):
                mlp_pieces.append(mk_l1(jc))
            for kc in range(KC):
                mlp_pieces.append(mk_l2(kc))
            mlp_pieces.append(head)

        g_sbs = {}

        def emit_transform(w):
            """Transpose g(w) and FiLM-multiply into its group's gmodT."""
            g_sb = g_sbs.pop(w)
            pt_t = pt.tile([128, H], dt.float32, name="pt_t")
            for hc in range(HC):
                nc.tensor.transpose(
                    pt_t[:, hc * 128:(hc + 1) * 128],
                    g_sb[:, hc * 128:(hc + 1) * 128],
                    iden_sb[:])
            gi = group_of[w]
            wi = w - gstarts[gi]
            gam, gmodT = gstate[gi]
            pt_v = pt_t[:].rearrange("p (c s) -> p c s", c=HC)
            gm_v = gmodT[:].rearrange("p c (g s) -> p c g s", g=GRP)
            ga_v = gam[:].rearrange("p c (g s) -> p c g s", g=GRP)
            nc.vector.tensor_mul(
                gm_v[:, :, wi, :], pt_v, ga_v[:, :, wi, :])

        mlp_done = set()

        def emit_mlp_group(grp_i):
            if grp_i in mlp_done:
                return
            mlp_done.add(grp_i)
            queue_mlp(gstarts[grp_i], gplan[grp_i] * SEG_W, grp_i)

        for w in range(windows):
            if w in gstarts:
                emit_gamma(group_of[w])

            # --- segment-sum for this window: accumulate [128 seg, H] ---
            pg_t = pg.tile([128, H], dt.float32)
            off = 0
            for blk in blocks:
                x_sb = xpool.tile([128, XT, H], x_dt)
                nc.sync.dma_start(
                    x_sb[:, 0:blk, :],
                    xp[w][:, off * H:(off + blk) * H]
                    .rearrange("p (c h) -> p c h", c=blk))
                # batched one-hot for the whole block: [128, blk, 128] fp8
                oh = ohpool.tile([128, XT, 128], x_dt)
                iotr_v = iotr_sb[:].rearrange("p (o s) -> p o s", o=1)
                brt_v = brt_sb[:, w * t_tiles + off: w * t_tiles + off + blk]
                brt_v = brt_v.rearrange("p (c o) -> p c o", o=1)
                in0, in1 = bass.broadcast_tensor_aps(iotr_v, brt_v)
                nc.vector.tensor_tensor(oh[:, 0:blk, :], in0, in1, is_eq)
                for gpair in range(blk // 2):
                    ti = off + 2 * gpair
                    nc.tensor.matmul(
                        pg_t[:],
                        oh[:, 2 * gpair:2 * gpair + 2, :],
                        x_sb[:, 2 * gpair:2 * gpair + 2, :],
                        start=(ti == 0), stop=(ti == t_tiles - 2),
                        perf_mode=DR)
                off += blk

            if w >= 1:
                emit_transform(w - 1)

            # evict g(w) on the DVE, emitted after the transposes of w-1 so
            # those transposes only wait on already-finished DVE work; the
            # copy itself runs during the next window's PE stream
            g_sb = spool.tile([128, H], dt.float32, tag="g", name="g_sb")
            nc.vector.tensor_copy(g_sb[:], pg_t[:])
            g_sbs[w] = g_sb

            if w == 0:
                emit_weight_dmas()
            if w >= 1:
                g1 = group_of[w - 1]
                if w - 1 == gstarts[g1] + gplan[g1] - 1:
                    emit_mlp_group(g1)
            for _ in range(3):
                if mlp_pieces:
                    mlp_pieces.pop(0)()

        emit_transform(windows - 1)
        for grp_i in range(len(gplan)):
            emit_mlp_group(grp_i)
        while mlp_pieces:
            mlp_pieces.pop(0)()

    nc.compile()
    return nc


def _sigma_delta_fp8(x: np.ndarray, batch: np.ndarray, n_segs: int) -> np.ndarray:
    """fp8-e4m3 quantization of x with per-(segment, h) error feedback.

    Within each segment the quantization errors telescope, so segment sums
    of the returned array match the fp32 sums to ~1 quantum.
    """
    starts = np.searchsorted(batch, np.arange(n_segs + 1))
    lens = np.diff(starts)
    L = int(lens.max())
    xq = np.empty(x.shape, dtype=F8)
    order = np.argsort(-lens, kind="stable")  # longest first: shrinking actives
    sorted_lens = lens[order]
    sorted_starts = starts[order]
    carry = np.zeros((n_segs, x.shape[1]), np.float32)
    for k in range(L):
        n_act = int(np.searchsorted(-sorted_lens, -k, side="left"))
        if n_act == 0:
            break
        rows = sorted_starts[:n_act] + k
        v = x[rows] + carry[:n_act]
        q = v.astype(F8)
        carry[:n_act] = v - q.astype(np.float32)
        xq[rows] = q
    return xq


def _balance_windows(batch: np.ndarray, n_segs: int, n_windows: int):
    """Greedy LPT assignment of segments to windows (128 segments each),
    equalizing node counts.  Returns (win_of_seg, slot_of_seg, t_tiles)."""
    counts = np.bincount(batch, minlength=n_segs)
    order = np.argsort(-counts, kind="stable")
    loads = np.zeros(n_windows, np.int64)
    nseg = np.zeros(n_windows, np.int64)
    win = np.empty(n_segs, np.int64)
    INF = 1 << 40
    for s in order:
        eligible = np.where(nseg < SEG_W, loads, INF)
        w = int(np.argmin(eligible))
        win[s] = w
        loads[w] += counts[s]
        nseg[w] += 1
    assert (nseg == SEG_W).all()
    # slot of each segment within its window (stable by segment id)
    o = np.argsort(win, kind="stable")
    slot = np.empty(n_segs, np.int64)
    slot[o] = np.arange(n_segs) - np.repeat(
        np.arange(n_windows) * SEG_W, SEG_W)
    t_tiles = max(2, 2 * int(-(-loads.max() // 256)))
    return win, slot, t_tiles


def prepare_core_inputs(
    x, batch, domain_emb, gamma_w, gamma_b, beta_w, beta_b,
    w1, b1, w2, b2, w3, b3,
    spc: int, n_cores: int, plan=None,
):
    """Slice/pad/pack the full inputs into one in_map per core.

    Returns (in_maps, seg_pos) where seg_pos[seg] is the segment's position
    in the permuted, concatenated output."""
    n_segs = spc * n_cores
    windows = spc // SEG_W
    n_win_tot = windows * n_cores

    batch = np.ascontiguousarray(np.asarray(batch).astype(np.int64))
    x = np.asarray(x, dtype=np.float32)

    if plan is None:
        plan = _balance_windows(batch, n_segs, n_win_tot)
    win, slot, t_tiles = plan
    npw = SEG_W * t_tiles

    w1_f = np.asarray(w1, np.float32)
    bw_ext = np.concatenate([np.asarray(beta_w, np.float32).T,
                             np.asarray(beta_b, np.float32)[None]], axis=0)  # [17, H]
    w1bw = bw_ext @ w1_f.T                                                   # [17, H]

    shared = {
        "gw": np.ascontiguousarray(_f32_to_bf16(
            np.concatenate([np.asarray(gamma_w, np.float32).T,
                            np.asarray(gamma_b, np.float32)[None]], axis=0))),
        "w1bw": np.ascontiguousarray(_f32_to_bf16(w1bw)),
        "w1t": np.ascontiguousarray(_f32_to_bf16(w1_f.T)),
        "w2t": np.ascontiguousarray(_f32_to_bf16(np.asarray(w2, np.float32).T)),
        "w3c": np.ascontiguousarray(
            _f32_to_bf16(np.asarray(w3, np.float32).reshape(H2 // 128, 128).T)),
        "b1c": np.ascontiguousarray(np.asarray(b1, np.float32).reshape(H // 128, 128).T),
        "b2c": np.ascontiguousarray(np.asarray(b2, np.float32).reshape(H2 // 128, 128).T),
        "b3c": np.asarray(b3, np.float32).reshape(1, 1),
        "iden": np.eye(128, dtype=np.float32),
        "iotr": np.tile(np.arange(128, dtype=np.float32), (128, 1)).astype(BF16),
    }

    xq_u8 = _sigma_delta_fp8(x, batch, n_segs).view(np.uint8)

    # permuted node order: grouped by window (stable, so per-segment runs stay
    # contiguous), with per-node window/slot ids
    node_win = win[batch]
    node_slot = slot[batch].astype(np.float32)
    order = np.argsort(node_win, kind="stable")
    wstarts = np.searchsorted(node_win[order], np.arange(n_win_tot + 1))

    # segment position in the permuted output
    seg_pos = win * SEG_W + slot

    dom = np.asarray(domain_emb, np.float32)
    dom_ext = np.concatenate([dom.T, np.ones((1, n_segs), np.float32)], axis=0)
    domP = np.empty((FD + 1, n_segs), np.float32)
    domP[:, seg_pos] = dom_ext

    in_maps = []
    for core in range(n_cores):
        xp_c = np.zeros((windows, npw, H), dtype=np.uint8)
        brt_c = np.full((windows, npw), -1024.0, dtype=np.float32)
        for wl in range(windows):
            wg = core * windows + wl
            ns = order[wstarts[wg]:wstarts[wg + 1]]
            cnt = len(ns)
            if cnt > npw:
                raise ValueError(f"window overflow: {cnt} > {npw}")
            if cnt == 0:
                continue
            xp_c[wl, :cnt] = xq_u8[ns]
            brt_c[wl, :cnt] = node_slot[ns]
        # [windows, npw, H] -> [windows, 128, t_tiles*H]: node c*128+p at
        # partition p, free slot (c, h)
        xp_c = np.ascontiguousarray(
            xp_c.reshape(windows, t_tiles, 128, H)
            .transpose(0, 2, 1, 3)
            .reshape(windows, 128, t_tiles * H)).view(F8)
        # [windows, npw] -> [128, windows*t_tiles]: brt[p, w*t_tiles+ti]
        brt_c = np.ascontiguousarray(
            brt_c.reshape(windows, t_tiles, 128).transpose(2, 0, 1)
            .reshape(128, windows * t_tiles).astype(BF16))
        domE_c = np.ascontiguousarray(
            _f32_to_bf16(domP[:, core * spc:(core + 1) * spc]))
        in_maps.append({"xp": xp_c, "brt": brt_c, "domE": domE_c, **shared})
    return in_maps, seg_pos, t_tiles


_PROGRAM_CACHE: dict = {}

# Set by test harnesses: request an NTFF trace and stash the raw results.
TRACE = False
LAST_RESULT = None


def kernel(**inputs) -> np.ndarray:
    x = np.asarray(inputs["x"], dtype=np.float32)
    batch = np.ascontiguousarray(np.asarray(inputs["batch"]).astype(np.int64))
    assert x.shape == (N_NODES, H), x.shape

    spc = B_SEGS // N_CORES

    in_maps, seg_pos, t_tiles = prepare_core_inputs(
        x, batch,
        inputs["domain_emb"], inputs["gamma_w"], inputs["gamma_b"],
        inputs["beta_w"], inputs["beta_b"],
        inputs["w1"], inputs["b1"], inputs["w2"], inputs["b2"],
        inputs["w3"], inputs["b3"],
        spc, N_CORES,
    )

    key = (spc, t_tiles, N_CORES)
    if key not in _PROGRAM_CACHE:
        _PROGRAM_CACHE[key] = build_program(spc, t_tiles, N_CORES)
    nc = _PROGRAM_CACHE[key]

    res = bass_utils.run_bass_kernel_spmd(
        nc, in_maps, core_ids=list(range(N_CORES)), trace=TRACE)
    global LAST_RESULT
    LAST_RESULT = res
    out_perm = np.concatenate(
        [res.results[c]["out"].reshape(-1) for c in range(N_CORES)])
    return np.ascontiguousarray(out_perm[seg_pos].astype(np.float32))
